# revision 8
# baseline (speedup 1.0000x reference)
"""TRN2 Bass kernel for OneLayerCNN: conv2d(4x4, stride 2, pad 2) + bias + ReLU.

Input  A_prev (64, 256, 256, 3) f32, W (4,4,3,16), b (1,1,1,16)
Output (64, 129*129*16) f32.

Data-parallel over 8 NeuronCores (8 images each). Per core:
  - rows of one image = 768 contiguous floats; loaded into SBUF tiles whose
    partitions are (row, img) instances split by row parity, img-minor.
  - PE transpose (is_transpose matmul vs identity) turns natural [instance,
    offset] windows into [offset, instance] tiles (K on partitions).
  - conv = per (h-block, w-block) 4 accumulating float32r matmuls:
    stationary lhsT = transposed-activation window [K<=121, M=120 instances],
    moving rhs = host-precomputed banded weights [121, 304 = 19 w' x 16 cout].
    Bias enters via a ones-row at K=120 on the fh=0 matmul.
  - ReLU on DVE eviction PSUM->SBUF, one contiguous-run DMA per h-block.
"""
import numpy as np
from contextlib import ExitStack

import concourse.bass as bass
import concourse.tile as tile
from concourse import mybir
from concourse.bass_utils import run_bass_kernel_spmd
import bass_rust

# ---------------- problem constants (hardcoded) ----------------
N_CORES = 8
IMG = 8              # images per core
H = 256
WID = 256
CIN = 3
F = 4
COUT = 16
HO = 129
WO = 129
RW = WID * CIN       # 768 floats per row
PADL = 6             # 2 pixels * 3ch left zero pad
RWP = 816            # padded row width (6 + 768 + 42)
NH_FULL = 15         # h' rows per full block
NB = 9               # 8 full blocks + 1 ragged (9 h')
WBLK = 19            # w' per w-block (B=0..5), B=6 computes 16, keeps 15
NWB = 7
KW = 120             # banded K window (6*18+12)
KB = 121             # K incl bias row
NMM = WBLK * COUT    # 304
OUTROW = WO * COUT   # 2064

DT_MM = mybir.dt.float32r   # matmul dtype knob (float32r | float32)
DT_F32 = mybir.dt.float32


def _split_multi_waits(nc):
    """walrus here accepts at most ONE sync wait per instruction; hoist
    extras onto NoOps inserted just before, same engine queue."""
    ctr = 0
    for f in nc.m.functions:
        for bb in f.blocks:
            insts = bb.instructions  # live list
            out = []
            changed = False
            for inst in insts:
                si = inst.sync_info
                if si is None:
                    out.append(inst)
                    continue
                waits = list(si.on_wait)
                if len(waits) > 1:
                    changed = True
                    for w in waits[:-1]:
                        ctr += 1
                        nop = mybir.InstNoOp(name=f"I-wsplit-{ctr}")
                        nop.engine = inst.engine
                        nop.sync_info = bass_rust.SyncInfo(
                            on_wait=[w], on_update=[])
                        out.append(nop)
                    inst.sync_info = bass_rust.SyncInfo(
                        on_wait=[waits[-1]], on_update=list(si.on_update))
                out.append(inst)
            if changed:
                insts[:] = out
    return nc


def _make_wband(W_arr, b_arr):
    """4 banded weight mats [121, 304]: wb[fh][6s+3fw+ci, 16s+co] = W[fh,fw,ci,co];
    wb[0][120, 16s+co] = b[co]."""
    wbs = []
    for fh in range(F):
        wb = np.zeros((KB, NMM), dtype=np.float32)
        for s in range(WBLK):
            for fw in range(F):
                for ci in range(CIN):
                    wb[6 * s + 3 * fw + ci, 16 * s:16 * s + 16] = \
                        W_arr[fh, fw, ci, :]
        if fh == 0:
            for s in range(WBLK):
                wb[120, 16 * s:16 * s + 16] = b_arr.reshape(-1)
        wbs.append(wb)
    return wbs


def _build_nc(dt_mm=DT_MM):
    nc = bass.Bass()
    a_in = nc.declare_dram_parameter("A", [IMG, H, RW], dt_mm, isOutput=False)
    wb_in = [nc.declare_dram_parameter(f"wb{fh}", [KB, NMM], dt_mm,
                                       isOutput=False) for fh in range(F)]
    id_in = nc.declare_dram_parameter("ident", [128, 128], dt_mm,
                                      isOutput=False)
    z_out = nc.declare_dram_parameter("Z", [IMG, HO, OUTROW], DT_F32,
                                      isOutput=True)

    # rows by parity: apar[p][re] = row 2*re+p of each image
    apar = a_in.rearrange("i (re two) c -> two re i c", two=2)

    with tile.TileContext(nc) as tc, ExitStack() as ctx:
        consts = ctx.enter_context(tc.tile_pool(name="consts", bufs=1))
        rpool = ctx.enter_context(tc.tile_pool(name="rows", bufs=2))
        tpool = ctx.enter_context(tc.tile_pool(name="tsb", bufs=4))
        opool = ctx.enter_context(tc.tile_pool(name="oacc", bufs=2))
        pt_pool = ctx.enter_context(
            tc.tile_pool(name="ptr", bufs=4, space="PSUM"))
        pc_pool = ctx.enter_context(
            tc.tile_pool(name="pconv", bufs=3, space="PSUM"))

        wb_sb = []
        for fh in range(F):
            t = consts.tile([KB, NMM], dt_mm, tag=f"wb{fh}", name=f"wb_sb{fh}")
            nc.sync.dma_start(out=t[:], in_=wb_in[fh][:])
            wb_sb.append(t)
        ident = consts.tile([128, 128], dt_mm, tag="ident")
        nc.sync.dma_start(out=ident[:], in_=id_in[:])

        for b in range(NB):
            h0 = NH_FULL * b
            nh = NH_FULL if b < NB - 1 else HO - NH_FULL * (NB - 1)  # 15 | 9
            nl = nh + 1          # parity rows needed: re = h0-1 .. h0+nh-1
            m = nh * IMG         # matmul M (120 | 72)

            rp = [rpool.tile([128, RWP], dt_mm, tag=f"rp{p}", name=f"rp{p}")
                  for p in range(2)]
            for p in range(2):
                # zero pads: left/right columns always; pad rows at edges
                nc.gpsimd.memset(rp[p][:, 0:PADL].bitcast(DT_F32), 0.0)
                nc.gpsimd.memset(rp[p][:, PADL + RW:RWP].bitcast(DT_F32), 0.0)
                l0, l1 = 0, nl
                if b == 0:
                    l0 = 1                      # re = -1 is a zero row
                    nc.gpsimd.memset(rp[p][0:8, :].bitcast(DT_F32), 0.0)
                if b == NB - 1:
                    l1 = nl - 1                 # re = 128 is a zero row
                    # 32-aligned base; rows below (nl-1)*8 are re-loaded by
                    # the DMA below, which follows in program order (WAW)
                    nc.gpsimd.memset(rp[p][64:128, :].bitcast(DT_F32), 0.0)
                re0 = h0 - 1 + l0
                src = apar[p, re0:re0 + (l1 - l0)]
                nc.sync.dma_start(
                    out=rp[p][l0 * 8:l1 * 8, PADL:PADL + RW], in_=src)

            oacc = opool.tile([128, OUTROW], DT_F32, tag="oacc")
            for B in range(NWB):
                win = 114 * B
                nmm = NMM if B < 6 else 256      # B=6: 16 w' (>=256 for f32r)
                ncols = NMM if B < 6 else 240    # evicted w' columns
                # both parities transposed into one PSUM tile, one eviction
                ptr = pt_pool.tile([KW, 256], dt_mm, tag="ptr", name="ptr")
                for p in range(2):
                    nc.tensor.transpose(
                        ptr[:, 128 * p:128 * (p + 1)],
                        rp[p][:, win:win + KW], ident[:])
                tsb = tpool.tile([KB, 256], dt_mm, tag="tsb", name="tsb")
                # bias ones row at partition 120 (even half): memset a
                # 32-aligned range first, the evict overwrites 96..119
                nc.gpsimd.memset(tsb[96:KB, 0:128].bitcast(DT_F32), 1.0)
                nc.vector.tensor_copy(tsb[0:KW, :], ptr[:])
                pc = pc_pool.tile([128, NMM], DT_F32, tag="pc")
                nc.tensor.matmul(pc[0:m, 0:nmm], tsb[0:KB, 0:m],
                                 wb_sb[0][0:KB, 0:nmm],
                                 start=True, stop=False)
                nc.tensor.matmul(pc[0:m, 0:nmm], tsb[0:KW, 128:128 + m],
                                 wb_sb[1][0:KW, 0:nmm],
                                 start=False, stop=False)
                nc.tensor.matmul(pc[0:m, 0:nmm], tsb[0:KW, 8:8 + m],
                                 wb_sb[2][0:KW, 0:nmm],
                                 start=False, stop=False)
                nc.tensor.matmul(pc[0:m, 0:nmm], tsb[0:KW, 136:136 + m],
                                 wb_sb[3][0:KW, 0:nmm],
                                 start=False, stop=True)
                # ReLU eviction: alternate ACT/DVE to balance engines
                if B % 2 == 0:
                    nc.scalar.activation(
                        oacc[0:m, 304 * B:304 * B + ncols], pc[0:m, 0:ncols],
                        mybir.ActivationFunctionType.Relu)
                else:
                    nc.vector.tensor_scalar_max(
                        oacc[0:m, 304 * B:304 * B + ncols],
                        pc[0:m, 0:ncols], 0.0)

            dst = z_out[:, h0:h0 + nh, :].rearrange("i j c -> j i c")
            nc.scalar.dma_start(out=dst, in_=oacc[0:m, :])

    _split_multi_waits(nc)
    return nc


_NC_CACHE = {}


def _get_nc(dt_mm=DT_MM):
    key = str(dt_mm)
    if key not in _NC_CACHE:
        _NC_CACHE[key] = _build_nc(dt_mm)
    return _NC_CACHE[key]


def kernel(A_prev, W, b, _trace=False, _dt=None):
    A_prev = np.ascontiguousarray(A_prev, dtype=np.float32)
    W = np.asarray(W, dtype=np.float32)
    b = np.asarray(b, dtype=np.float32)
    wbs = _make_wband(W, b)
    ident = np.eye(128, dtype=np.float32)

    nc = _get_nc(_dt or DT_MM)
    in_maps = []
    for c in range(N_CORES):
        shard = A_prev[c * IMG:(c + 1) * IMG].reshape(IMG, H, RW)
        m = {"A": shard, "ident": ident}
        for fh in range(F):
            m[f"wb{fh}"] = wbs[fh]
        in_maps.append(m)

    res = run_bass_kernel_spmd(nc, in_maps, list(range(N_CORES)),
                               trace=_trace)
    out = np.concatenate([res.results[c]["Z"].reshape(IMG, -1)
                          for c in range(N_CORES)], axis=0)
    if _trace:
        return out, res
    return out
